# revision 15
# baseline (speedup 1.0000x reference)
"""DIEN (Deep Interest Evolution Network) Bass/Tile kernel for Trainium2.

Strategy: pure data parallel over batch. Each of the 8 NeuronCores gets
B_local = 128 batch rows; embedding tables and all weights are replicated.

On-chip layout is feature-major: [feature -> partitions, batch -> free].
Embedding tables are held in bf16 (padded so rows are 128-wide), gathered
rows go through the DMA xbar transpose into feature-major bf16 tiles, and
the GRU input projections (gx) are computed in batched bf16 matmuls
directly into the same PSUM tiles that the sequential scan then
accumulates gh = Whh @ h onto (matmul start=False), so no elementwise add
is needed for gx + gh. Sequence masking is folded into the z gate by
adding 40*(1-mask) to the z pre-activation (sigmoid saturates to exactly
1.0f, freezing h), and interests are masked in one batched multiply per
chunk. Biases are exact and free: GRU input-side biases ride on an
appended ones-column of the category table; per-feature biases use the
scalar/bias slots of activation / scalar_tensor_tensor ops. The
recurrent state math stays in fp32.
"""

import os
import sys

for _p in ("/opt/trn_rl_repo", "/root/.axon_site/_ro/trn_rl_repo"):
    if os.path.isdir(_p) and _p not in sys.path:
        sys.path.insert(0, _p)

import numpy as np
from ml_dtypes import bfloat16 as np_bf16

import concourse.bacc as bacc
import concourse.bass as bass
import concourse.mybir as mybir
import concourse.tile as tile
from concourse.bass import IndirectOffsetOnAxis
from concourse.bass_utils import run_bass_kernel_spmd

F32 = mybir.dt.float32
BF16 = mybir.dt.bfloat16
I32 = mybir.dt.int32
AF = mybir.ActivationFunctionType
OP = mybir.AluOpType

B, S, D, H, F = 1024, 200, 128, 128, 10
NU, NI, NC = 100000, 100000, 1000
DC = D // 2          # 64
DH = D + DC          # 192
NCORES = 8
BL = B // NCORES     # 128 batch rows per core
TC = 4               # timesteps per pipeline chunk
KCH_HOST = [D, D, DC, H, H, F]

_BUILT = {}


def _ap3(base, off, mid_step, mid_n, inner_n):
    """3D view [P, mid_n, inner_n] of a 2D tile AP at free-offset off."""
    a = base[:, off:off + 1]
    return bass.AP(a.tensor, a.offset, [a.ap[0], [mid_step, mid_n], [1, inner_n]])


def _build(ss):
    """Build + compile the single-core module for ss timesteps."""
    nc = bacc.Bacc("TRN2", target_bir_lowering=False, debug=False)
    nch = ss // TC

    def din(name, shape, dt=F32):
        return nc.dram_tensor(name, list(shape), dt, kind="ExternalInput").ap()

    # per-core data
    uid = din("uid", [BL, 1], I32)
    aid = din("aid", [BL, 1], I32)
    cid = din("cid", [BL, 1], I32)
    hitems = din("hitems", [BL, ss], I32)
    hcats = din("hcats", [BL, ss], I32)
    mrow = din("mrow", [1, ss * BL], BF16)   # mask, (s, b) raveled
    omrow = din("omrow", [1, ss * BL], BF16)  # 40*(1-mask)
    featT = din("featT", [F, BL], BF16)
    # tables (replicated, bf16; cat table padded to 128 with ones column @64)
    utab = din("utab", [NU, D], BF16)
    itab = din("itab", [NI, D], BF16)
    ctab = din("ctab", [NC, 128], BF16)
    # weights
    wih_i = din("wih_i", [D, 3 * H], BF16)    # gru_Wih[:, :D].T
    wih_c = din("wih_c", [128, 3 * H], BF16)  # [gru_Wih[:, D:].T ; bias row; 0]
    whh = din("whh", [H, 3 * H])              # gru_Whh.T (fp32)
    bhhn = din("bhhn", [H, 1])
    w1i = din("w1i", [H, 80], BF16)
    w1ti = din("w1ti", [D, 80], BF16)
    w1tc = din("w1tc", [DC, 80], BF16)
    b1 = din("b1", [80, 1])
    w2 = din("w2", [80, 40], BF16)
    b2 = din("b2", [40, 1])
    w3r = din("w3r", [40, 128], BF16)
    b3r = din("b3r", [128, 1])
    wir = din("wir", [H, H], BF16)
    wiha = din("wiha", [H, H], BF16)
    whr = din("whr", [H, H])
    whha = din("whha", [H, H])
    br = din("br", [H, 1])
    bh = din("bh", [H, 1])
    fc1k = [din(f"fc1k{j}", [k, 256], F32 if j == 3 else BF16)
            for j, k in enumerate(KCH_HOST)]
    fb1a = din("fb1a", [128, 1])
    fb1b = din("fb1b", [128, 1])
    fc2a = din("fc2a", [128, 128], BF16)
    fc2b = din("fc2b", [128, 128], BF16)
    fb2 = din("fb2", [128, 1])
    fc3 = din("fc3", [128, 64], BF16)
    fb3 = din("fb3", [64, 1])
    fc4 = din("fc4", [64, 1], BF16)
    fb4 = din("fb4", [1, 1])
    ones_row = din("ones_row", [1, BL], BF16)
    identb = din("identb", [128, 128], BF16)
    id80 = din("id80", [80, 80], BF16)

    out = nc.dram_tensor("out", [1, BL], F32, kind="ExternalOutput").ap()

    with tile.TileContext(nc) as tc:
        with tc.tile_pool(name="persist", bufs=1) as pp:
            # ---- persistent SBUF tensors ----
            def load(name, ap_in, shape, dt=F32):
                t = pp.tile(shape, dt, name=name)
                nc.sync.dma_start(out=t[:], in_=ap_in[:])
                return t

            wih_i_s = load("wih_i_s", wih_i, [D, 3 * H], BF16)
            wih_c_s = load("wih_c_s", wih_c, [128, 3 * H], BF16)
            whh_s = load("whh_s", whh, [H, 3 * H])
            bhhn_s = load("bhhn_s", bhhn, [H, 1])
            w1i_s = load("w1i_s", w1i, [H, 80], BF16)
            w1ti_s = load("w1ti_s", w1ti, [D, 80], BF16)
            w1tc_s = load("w1tc_s", w1tc, [DC, 80], BF16)
            b1_s = load("b1_s", b1, [80, 1])
            w2_s = load("w2_s", w2, [80, 40], BF16)
            b2_s = load("b2_s", b2, [40, 1])
            w3r_s = load("w3r_s", w3r, [40, 128], BF16)
            b3r_s = load("b3r_s", b3r, [128, 1])
            wir_s = load("wir_s", wir, [H, H], BF16)
            wiha_s = load("wiha_s", wiha, [H, H], BF16)
            whr_s = load("whr_s", whr, [H, H])
            whha_s = load("whha_s", whha, [H, H])
            br_s = load("br_s", br, [H, 1])
            bh_s = load("bh_s", bh, [H, 1])
            fc1k_s = [load(f"fc1k{j}_s", fc1k[j], [KCH_HOST[j], 256],
                           F32 if j == 3 else BF16) for j in range(6)]
            fb1a_s = load("fb1a_s", fb1a, [128, 1])
            fb1b_s = load("fb1b_s", fb1b, [128, 1])
            fc2a_s = load("fc2a_s", fc2a, [128, 128], BF16)
            fc2b_s = load("fc2b_s", fc2b, [128, 128], BF16)
            fb2_s = load("fb2_s", fb2, [128, 1])
            fc3_s = load("fc3_s", fc3, [128, 64], BF16)
            fb3_s = load("fb3_s", fb3, [64, 1])
            fc4_s = load("fc4_s", fc4, [64, 1], BF16)
            fb4_s = load("fb4_s", fb4, [1, 1])
            ones_s = load("ones_s", ones_row, [1, BL], BF16)
            featT_s = load("featT_s", featT, [F, BL], BF16)
            ident_s = load("ident_s", identb, [128, 128], BF16)
            id80_s = load("id80_s", id80, [80, 80], BF16)

            h0 = pp.tile([128, BL], F32, name="h0")
            nc.gpsimd.memset(h0[:], 0)

            userT = pp.tile([D, BL], BF16, name="userT")
            itemT = pp.tile([D, BL], BF16, name="itemT")
            catT = pp.tile([128, BL], BF16, name="catT")
            attc = pp.tile([80, BL], BF16, name="attc")  # W1t @ tgt + b1
            ibig = pp.tile([128, ss * BL], BF16, name="ibig")  # masked interests
            hau = pp.tile([128, BL], F32, name="hau")  # AUGRU state (final)

            # ---- phase 1: target/user embedding lookups ----
            with tc.tile_pool(name="p1s", bufs=1) as p1:
                g_u = p1.tile([BL, D], BF16, name="g_u")
                g_a = p1.tile([BL, D], BF16, name="g_a")
                g_c = p1.tile([BL, 128], BF16, name="g_c")
                for g, table, idx in ((g_u, utab, uid), (g_a, itab, aid), (g_c, ctab, cid)):
                    idx_s = p1.tile([BL, 1], I32, name="idx_s", tag="idx_s", bufs=3)
                    nc.sync.dma_start(out=idx_s[:], in_=idx[:])
                    nc.gpsimd.indirect_dma_start(
                        out=g[:], out_offset=None, in_=table[:],
                        in_offset=IndirectOffsetOnAxis(ap=idx_s[:, :1], axis=0),
                    )
                for g, dst in ((g_u, userT), (g_a, itemT), (g_c, catT)):
                    nc.sync.dma_start(out=dst[:], in_=g[:], transpose=True)
                # attention constant: W1t @ [itemT; catT] + b1
                with tc.tile_pool(name="p1p", bufs=1, space="PSUM") as q1:
                    psc = q1.tile([80, BL], F32, name="psc")
                    nc.tensor.matmul(out=psc[:], lhsT=w1ti_s[:], rhs=itemT[:], start=True, stop=False)
                    nc.tensor.matmul(out=psc[:], lhsT=w1tc_s[:], rhs=catT[:DC, :], start=False, stop=True)
                    nc.scalar.activation(out=attc[:], in_=psc[:], func=AF.Identity, bias=b1_s[:, :1])

            # ---- phase 2: GRU pass (gather -> xbar transpose -> gx -> scan) ----
            with (
                tc.tile_pool(name="p2s", bufs=1) as p2,
                tc.tile_pool(name="p2p", bufs=1, space="PSUM") as q2,
            ):
                h_prev = h0[:]
                for c in range(nch):
                    t0 = c * TC
                    idx_i = p2.tile([BL, TC], I32, name="idx_i", tag="idx_i", bufs=3)
                    idx_c = p2.tile([BL, TC], I32, name="idx_c", tag="idx_c", bufs=3)
                    nc.sync.dma_start(out=idx_i[:], in_=hitems[:, t0:t0 + TC])
                    nc.sync.dma_start(out=idx_c[:], in_=hcats[:, t0:t0 + TC])
                    gi = p2.tile([BL, TC * D], BF16, name="gi", tag="gi", bufs=3)
                    gc = p2.tile([BL, TC * 128], BF16, name="gc", tag="gc", bufs=3)
                    nc.gpsimd.indirect_dma_start(
                        out=gi[:], out_offset=None, in_=itab[:],
                        in_offset=IndirectOffsetOnAxis(ap=idx_i[:, :TC], axis=0))
                    nc.gpsimd.indirect_dma_start(
                        out=gc[:], out_offset=None, in_=ctab[:],
                        in_offset=IndirectOffsetOnAxis(ap=idx_c[:, :TC], axis=0))
                    hti = p2.tile([D, TC * BL], BF16, name="hti", tag="hti", bufs=2)
                    htc = p2.tile([128, TC * BL], BF16, name="htc", tag="htc", bufs=2)
                    for t in range(TC):
                        eng = nc.sync if t % 2 == 0 else nc.scalar
                        eng.dma_start(out=hti[:, t * BL:(t + 1) * BL],
                                      in_=gi[:, t * D:(t + 1) * D], transpose=True)
                        eng2 = nc.scalar if t % 2 == 0 else nc.sync
                        eng2.dma_start(out=htc[:, t * BL:(t + 1) * BL],
                                       in_=gc[:, t * 128:(t + 1) * 128], transpose=True)

                    m1 = p2.tile([1, TC * BL], BF16, name="m1", tag="m1", bufs=2)
                    om1 = p2.tile([1, TC * BL], BF16, name="om1", tag="om1", bufs=2)
                    nc.sync.dma_start(out=m1[:], in_=mrow[:, t0 * BL:(t0 + TC) * BL])
                    nc.sync.dma_start(out=om1[:], in_=omrow[:, t0 * BL:(t0 + TC) * BL])

                    # batched gx into PSUM (scan accumulates on top)
                    # pg_rz: [r gates | z gates], one psum bank per half
                    pg_rz = q2.tile([128, 2 * TC * BL], F32, name="pg_rz", tag="pg_rz", bufs=2)
                    pg_n = q2.tile([128, TC * BL], F32, name="pg_n", tag="pg_n", bufs=1)
                    NTB = TC * BL
                    for g, (pg, o) in enumerate(((pg_rz, 0), (pg_rz, NTB), (pg_n, 0))):
                        dst = pg[:, o:o + NTB]
                        nc.tensor.matmul(out=dst, skip_group_check=True,
                                         lhsT=wih_i_s[:, g * H:(g + 1) * H],
                                         rhs=hti[:], start=True, stop=False)
                        nc.tensor.matmul(out=dst, skip_group_check=True,
                                         lhsT=wih_c_s[:, g * H:(g + 1) * H],
                                         rhs=htc[:], start=False, stop=False)
                    # z-gate mask bias: + 40*(1-m) saturates sigmoid -> 1
                    nc.tensor.matmul(out=pg_rz[:, NTB:2 * NTB], skip_group_check=True,
                                     lhsT=ones_s[:], rhs=om1[:], start=False, stop=False)
                    # partition-replicated mask for interests
                    mrep = q2.tile([128, TC * BL], F32, name="mrep", tag="mrep", bufs=2)
                    nc.tensor.matmul(out=mrep[:], lhsT=ones_s[:], rhs=m1[:],
                                     start=True, stop=True)

                    hc = p2.tile([128, TC * BL], F32, name="hc", tag="hc", bufs=2)
                    for t in range(TC):
                        sl = slice(t * BL, (t + 1) * BL)
                        slz = slice(NTB + t * BL, NTB + (t + 1) * BL)
                        last = t == TC - 1
                        nc.tensor.matmul(out=pg_rz[:, sl], skip_group_check=True,
                                         lhsT=whh_s[:, 0:H], rhs=h_prev,
                                         start=False, stop=last)
                        nc.tensor.matmul(out=pg_rz[:, slz], skip_group_check=True,
                                         lhsT=whh_s[:, H:2 * H], rhs=h_prev,
                                         start=False, stop=last)
                        ngh = q2.tile([128, BL], F32, name="ngh", tag="ngh", bufs=1)
                        nc.tensor.matmul(out=ngh[:], lhsT=whh_s[:, 2 * H:3 * H],
                                         rhs=h_prev, start=True, stop=True)
                        # fused sigmoid over [r_t | z_t] (strided 2-bank read)
                        rs = p2.tile([128, 2 * BL], F32, name="rs", tag="rs", bufs=2)
                        src = _ap3(pg_rz, t * BL, NTB, 2, BL)
                        dst = _ap3(rs, 0, BL, 2, BL)
                        nc.scalar.activation(out=dst, in_=src, func=AF.Sigmoid)
                        t1 = p2.tile([128, BL], BF16, name="t1", tag="t1", bufs=2)
                        # t1 = (ngh + bhh_n) * r
                        nc.vector.scalar_tensor_tensor(
                            out=t1[:], in0=ngh[:], scalar=bhhn_s[:, :1], in1=rs[:, 0:BL],
                            op0=OP.add, op1=OP.mult)
                        nc.tensor.matmul(out=pg_n[:, sl], skip_group_check=True,
                                         lhsT=ident_s[:], rhs=t1[:],
                                         start=False, stop=last)
                        nn = p2.tile([128, BL], F32, name="nn", tag="nn", bufs=2)
                        nc.scalar.activation(out=nn[:], in_=pg_n[:, sl], func=AF.Tanh)
                        dd_ = p2.tile([128, BL], F32, name="dd_", tag="dd_", bufs=2)
                        # d = h - n
                        nc.vector.scalar_tensor_tensor(
                            out=dd_[:], in0=nn[:], scalar=-1.0, in1=h_prev,
                            op0=OP.mult, op1=OP.add)
                        ee = p2.tile([128, BL], F32, name="ee", tag="ee", bufs=2)
                        nc.vector.tensor_tensor(out=ee[:], in0=rs[:, BL:2 * BL], in1=dd_[:], op=OP.mult)
                        nc.vector.tensor_tensor(out=hc[:, sl], in0=nn[:], in1=ee[:], op=OP.add)
                        h_prev = hc[:, sl]
                    # masked interests for this chunk (bf16)
                    nc.vector.tensor_tensor(out=ibig[:, t0 * BL:(t0 + TC) * BL],
                                            in0=hc[:], in1=mrep[:], op=OP.mult)

            # ---- phase 3: attention + AUGRU pass ----
            with (
                tc.tile_pool(name="p3s", bufs=1) as p3,
                tc.tile_pool(name="p3p", bufs=1, space="PSUM") as q3,
            ):
                ha_prev = h0[:]
                attc_bc = bass.AP(attc[:].tensor, attc[:].offset,
                                  [attc[:].ap[0], [0, TC], [1, BL]])
                for c in range(nch):
                    t0 = c * TC
                    isl = ibig[:, t0 * BL:(t0 + TC) * BL]
                    pgr = q3.tile([128, TC * BL], F32, name="pgr", tag="pgr", bufs=2)
                    pgn = q3.tile([128, TC * BL], F32, name="pgn", tag="pgn", bufs=2)
                    nc.tensor.matmul(out=pgr[:], skip_group_check=True, lhsT=wir_s[:], rhs=isl, start=True, stop=False)
                    nc.tensor.matmul(out=pgn[:], skip_group_check=True, lhsT=wiha_s[:], rhs=isl, start=True, stop=False)
                    pa1 = q3.tile([80, TC * BL], F32, name="pa1", tag="pa1", bufs=1)
                    nc.tensor.matmul(out=pa1[:], lhsT=w1i_s[:], rhs=isl, start=True, stop=False)
                    nc.tensor.matmul(out=pa1[:], lhsT=id80_s[:], rhs=attc_bc,
                                     start=False, stop=True)
                    a1s = p3.tile([80, TC * BL], BF16, name="a1s", tag="a1s", bufs=2)
                    nc.scalar.activation(out=a1s[:], in_=pa1[:], func=AF.Relu)
                    pa2 = q3.tile([40, TC * BL], F32, name="pa2", tag="pa2", bufs=1)
                    nc.tensor.matmul(out=pa2[:], lhsT=w2_s[:], rhs=a1s[:], start=True, stop=True)
                    a2s = p3.tile([40, TC * BL], BF16, name="a2s", tag="a2s", bufs=2)
                    nc.scalar.activation(out=a2s[:], in_=pa2[:], func=AF.Relu, bias=b2_s[:, :1])
                    pa3 = q3.tile([128, TC * BL], F32, name="pa3", tag="pa3", bufs=2)
                    nc.tensor.matmul(out=pa3[:], lhsT=w3r_s[:], rhs=a2s[:], start=True, stop=True)
                    a3s = p3.tile([128, TC * BL], BF16, name="a3s", tag="a3s", bufs=2)
                    nc.scalar.activation(out=a3s[:], in_=pa3[:], func=AF.Sigmoid, bias=b3r_s[:, :1])
                    for t in range(TC):
                        sl = slice(t * BL, (t + 1) * BL)
                        last = t == TC - 1
                        nc.tensor.matmul(out=pgr[:, sl], skip_group_check=True, lhsT=whr_s[:], rhs=ha_prev,
                                         start=False, stop=last)
                        ss_ = p3.tile([128, BL], F32, name="ss_", tag="ss_", bufs=2)
                        nc.scalar.activation(out=ss_[:], in_=pgr[:, sl], func=AF.Sigmoid,
                                             bias=br_s[:, :1])
                        uu = p3.tile([128, BL], F32, name="uu", tag="uu", bufs=2)
                        nc.vector.tensor_tensor(out=uu[:], in0=a3s[:, sl], in1=ss_[:], op=OP.mult)
                        hu = p3.tile([128, BL], F32, name="hu", tag="hu", bufs=2)
                        nc.vector.tensor_tensor(out=hu[:], in0=ha_prev, in1=uu[:], op=OP.mult)
                        nc.tensor.matmul(out=pgn[:, sl], skip_group_check=True, lhsT=whha_s[:], rhs=hu[:],
                                         start=False, stop=last)
                        ht_ = p3.tile([128, BL], F32, name="ht_", tag="ht_", bufs=2)
                        nc.scalar.activation(out=ht_[:], in_=pgn[:, sl], func=AF.Tanh,
                                             bias=bh_s[:, :1])
                        qq = p3.tile([128, BL], F32, name="qq", tag="qq", bufs=2)
                        nc.vector.tensor_tensor(out=qq[:], in0=uu[:], in1=ht_[:], op=OP.mult)
                        s2 = p3.tile([128, BL], F32, name="s2", tag="s2", bufs=2)
                        # s2 = h - hu
                        nc.vector.scalar_tensor_tensor(
                            out=s2[:], in0=hu[:], scalar=-1.0, in1=ha_prev,
                            op0=OP.mult, op1=OP.add)
                        is_last = (c == nch - 1 and t == TC - 1)
                        dst = hau if is_last else p3.tile([128, BL], F32, name="han", tag="han", bufs=2)
                        nc.vector.tensor_tensor(out=dst[:], in0=s2[:], in1=qq[:], op=OP.add)
                        ha_prev = dst[:]

            # ---- phase 4: final MLP ----
            with (
                tc.tile_pool(name="p4s", bufs=1) as p4,
                tc.tile_pool(name="p4p", bufs=1, space="PSUM") as q4,
            ):
                ilast = ibig[:, (ss - 1) * BL:ss * BL]
                rhs_list = [userT[:], itemT[:], catT[:DC, :], hau[:], ilast, featT_s[:]]
                pf1a = q4.tile([128, BL], F32, name="pf1a")
                pf1b = q4.tile([128, BL], F32, name="pf1b")
                for mi, (pf, mlo) in enumerate(((pf1a, 0), (pf1b, 128))):
                    for j in range(6):
                        nc.tensor.matmul(out=pf[:], lhsT=fc1k_s[j][:, mlo:mlo + 128],
                                         rhs=rhs_list[j], start=(j == 0), stop=(j == 5))
                x1a = p4.tile([128, BL], BF16, name="x1a")
                x1b = p4.tile([128, BL], BF16, name="x1b")
                nc.scalar.activation(out=x1a[:], in_=pf1a[:], func=AF.Relu, bias=fb1a_s[:, :1])
                nc.scalar.activation(out=x1b[:], in_=pf1b[:], func=AF.Relu, bias=fb1b_s[:, :1])
                pf2 = q4.tile([128, BL], F32, name="pf2")
                nc.tensor.matmul(out=pf2[:], lhsT=fc2a_s[:], rhs=x1a[:], start=True, stop=False)
                nc.tensor.matmul(out=pf2[:], lhsT=fc2b_s[:], rhs=x1b[:], start=False, stop=True)
                x2 = p4.tile([128, BL], BF16, name="x2")
                nc.scalar.activation(out=x2[:], in_=pf2[:], func=AF.Relu, bias=fb2_s[:, :1])
                pf3 = q4.tile([64, BL], F32, name="pf3")
                nc.tensor.matmul(out=pf3[:], lhsT=fc3_s[:], rhs=x2[:], start=True, stop=True)
                x3 = p4.tile([64, BL], BF16, name="x3")
                nc.scalar.activation(out=x3[:], in_=pf3[:], func=AF.Relu, bias=fb3_s[:, :1])
                pf4 = q4.tile([1, BL], F32, name="pf4")
                nc.tensor.matmul(out=pf4[:], lhsT=fc4_s[:], rhs=x3[:], start=True, stop=True)
                y = p4.tile([1, BL], F32, name="y")
                nc.scalar.activation(out=y[:], in_=pf4[:], func=AF.Sigmoid, bias=fb4_s[:1, :1])
                nc.sync.dma_start(out=out[:], in_=y[:])

    nc.compile()
    return nc


def get_module(ss=S):
    if ss not in _BUILT:
        _BUILT[ss] = _build(ss)
    return _BUILT[ss]


def host_prep(inputs, ss=S):
    """Build the 8 per-core input maps from full inputs."""
    f32 = np.float32
    gi = {k: np.asarray(v) for k, v in inputs.items()}
    gru_Wih, gru_Whh = gi["gru_Wih"].astype(f32), gi["gru_Whh"].astype(f32)
    gru_bih, gru_bhh = gi["gru_bih"].astype(f32), gi["gru_bhh"].astype(f32)

    ctab = np.zeros((NC, 128), f32)
    ctab[:, :DC] = gi["cat_table"].astype(f32)
    ctab[:, DC] = 1.0
    bias_row = gru_bih + np.concatenate([gru_bhh[:H], gru_bhh[H:2 * H], np.zeros(H, f32)])
    wih_c = np.zeros((128, 3 * H), f32)
    wih_c[:DC] = gru_Wih[:, D:].T
    wih_c[DC] = bias_row

    att_W1 = gi["att_W1"].astype(f32)

    def bf(x):
        return np.ascontiguousarray(np.asarray(x, f32).astype(np_bf16))

    shared = dict(
        utab=bf(gi["user_table"]),
        itab=bf(gi["item_table"]),
        ctab=bf(ctab),
        wih_i=bf(gru_Wih[:, :D].T),
        wih_c=bf(wih_c),
        whh=np.ascontiguousarray(gru_Whh.T),
        bhhn=np.ascontiguousarray(gru_bhh[2 * H:].reshape(H, 1)),
        w1i=bf(att_W1[:, :H].T),
        w1ti=bf(att_W1[:, H:H + D].T),
        w1tc=bf(att_W1[:, H + D:].T),
        b1=np.ascontiguousarray(gi["att_b1"].reshape(80, 1), f32),
        w2=bf(gi["att_W2"].T),
        b2=np.ascontiguousarray(gi["att_b2"].reshape(40, 1), f32),
        w3r=bf(np.tile(gi["att_W3"].astype(f32).T, (1, 128))),
        b3r=np.full((128, 1), gi["att_b3"][0], f32),
        wir=bf(gi["au_Wir"].T),
        wiha=bf(gi["au_Wih"].T),
        whr=np.ascontiguousarray(gi["au_Whr"].astype(f32).T),
        whha=np.ascontiguousarray(gi["au_Whh"].astype(f32).T),
        br=np.ascontiguousarray(gi["au_br"].reshape(H, 1), f32),
        bh=np.ascontiguousarray(gi["au_bh"].reshape(H, 1), f32),
        fb1a=np.ascontiguousarray(gi["fc1_b"][:128].reshape(128, 1), f32),
        fb1b=np.ascontiguousarray(gi["fc1_b"][128:].reshape(128, 1), f32),
        fb2=np.ascontiguousarray(gi["fc2_b"].reshape(128, 1), f32),
        fc3=bf(gi["fc3_W"].T),
        fb3=np.ascontiguousarray(gi["fc3_b"].reshape(64, 1), f32),
        fc4=bf(gi["fc4_W"].T),
        fb4=np.ascontiguousarray(gi["fc4_b"].reshape(1, 1), f32),
        ones_row=np.ones((1, BL), np_bf16),
        identb=np.eye(128, dtype=np_bf16),
        id80=np.eye(80, dtype=np_bf16),
    )
    fc1_W = gi["fc1_W"].astype(f32)
    bounds = np.cumsum([0] + KCH_HOST)
    for j in range(6):
        blk = fc1_W[:, bounds[j]:bounds[j + 1]].T
        shared[f"fc1k{j}"] = np.ascontiguousarray(blk) if j == 3 else bf(blk)
    fc2_W = gi["fc2_W"].astype(f32)
    shared["fc2a"] = bf(fc2_W[:, :128].T)
    shared["fc2b"] = bf(fc2_W[:, 128:].T)

    lens = np.maximum(gi["seq_lens"].astype(np.int64), 1)
    mask_full = (np.arange(ss)[:, None] < lens[None, :]).astype(f32)  # [ss, B]

    in_maps = []
    for c in range(NCORES):
        bs = slice(c * BL, (c + 1) * BL)
        m = np.ascontiguousarray(mask_full[:, bs])  # [ss, BL]
        im = dict(shared)
        im.update(
            uid=np.ascontiguousarray(gi["user_ids"][bs].reshape(BL, 1), np.int32),
            aid=np.ascontiguousarray(gi["article_ids"][bs].reshape(BL, 1), np.int32),
            cid=np.ascontiguousarray(gi["category_ids"][bs].reshape(BL, 1), np.int32),
            hitems=np.ascontiguousarray(gi["hist_items"][bs, :ss], np.int32),
            hcats=np.ascontiguousarray(gi["hist_cats"][bs, :ss], np.int32),
            mrow=bf(m.reshape(1, ss * BL)),
            omrow=bf((40.0 * (1.0 - m)).reshape(1, ss * BL)),
            featT=bf(gi["features"][bs].T),
        )
        in_maps.append(im)
    return in_maps


def kernel(**inputs):
    nc = get_module(S)
    in_maps = host_prep(inputs, S)
    res = run_bass_kernel_spmd(nc, in_maps, core_ids=list(range(NCORES)))
    outs = [res.results[c]["out"].reshape(BL, 1) for c in range(NCORES)]
    return np.concatenate(outs, 0).astype(np.float32)
